# revision 1
# baseline (speedup 1.0000x reference)
"""Trainium2 Bass kernel for nn_ModelA_ViT: 8-layer ViT encoder (D=1024, 16 heads,
2D RoPE, RMSNorm, GELU-tanh MLP) over 4x16x64x64 input, output [4, 1024, 1024].

Sharding: sequence-parallel over (batch, token-half): core c owns batch c//2,
tokens [512*(c%2), 512*(c%2)+512). Per layer, each core computes q/k/v for its
512 tokens, all-gathers rope'd K and V across all 8 cores through DRAM, and
runs attention rows for its own tokens against the full 1024-token K/V of its
batch (shards selected with partition-id-based dynamic DMA offsets).

Layout: residual stream is [D on partitions (8x128), tokens on free (512)]
("layout B") end-to-end - matches the [N, D, L] output, so no transposes are
ever needed. RMSNorm per-token stats use a ones-vector matmul partition-reduce
plus a K=1 matmul broadcast. RoPE rotate-half is one DVE stream_shuffle.
Softmax runs without max-subtraction (scores are provably small for this
model); denominators come free from a ones-column appended to V's stationary
operand. All matmuls use float32r (full PE rate, fp32 PSUM accumulation).
"""

import sys

sys.path.insert(0, "/opt/trn_rl_repo")

import os

import numpy as np

D = 1024
HEADS = 16
DH = 64
DEPTH = int(os.environ.get("VIT_LAYERS", "8"))
HID = 4096
NB = 4
L = 1024
T = 512  # tokens per core
NCORES = 8
EPS = 1e-6
FACT = 2

_cache = {}

def _env(name, dflt):
    return int(os.environ.get(name, str(dflt)))

WG_B = _env("VIT_WG", 12)
ACC_B = _env("VIT_ACC", 4)
SC_B = _env("VIT_SC", 2)
P_B = _env("VIT_P", 3)
KST_B = _env("VIT_KST", 2)
SQ_B = _env("VIT_SQ", 3)
RT1_B = _env("VIT_RT1", 1)
RT2_B = _env("VIT_RT2", 1)
RT3_B = _env("VIT_RT3", 1)
OACC_B = _env("VIT_OACC", 2)
KF_B = _env("VIT_KF", 1)
H_B = _env("VIT_H", 1)


def _build():
    import concourse.bass as bass
    import concourse.bacc as bacc
    import concourse.mybir as mybir
    import concourse.tile as tile

    f32 = mybir.dt.float32
    f32r = mybir.dt.float32r
    AF = mybir.ActivationFunctionType
    ALU = mybir.AluOpType
    ds = bass.ds

    nc = bacc.Bacc("TRN2", target_bir_lowering=False, debug=False, num_devices=NCORES)

    patchesT_d = nc.dram_tensor("patchesT", [64, T], f32r, kind="ExternalInput")
    merge_wT_d = nc.dram_tensor("merge_wT", [64, D], f32r, kind="ExternalInput")
    qkvT_d = nc.dram_tensor("qkvT", [DEPTH, D, 3 * D], f32r, kind="ExternalInput")
    projT_d = nc.dram_tensor("projT", [DEPTH, D, D], f32r, kind="ExternalInput")
    fc1T_d = nc.dram_tensor("fc1T", [DEPTH, D, HID], f32r, kind="ExternalInput")
    fc2T_d = nc.dram_tensor("fc2T", [DEPTH, HID, D], f32r, kind="ExternalInput")
    emat2_d = nc.dram_tensor("emat2", [2, 128], f32r, kind="ExternalInput")
    cos_d = nc.dram_tensor("cos_t", [128, T], f32, kind="ExternalInput")
    sinm_d = nc.dram_tensor("sinm_t", [128, T], f32, kind="ExternalInput")
    z_d = nc.dram_tensor("z", [D, T], f32, kind="ExternalOutput")

    SHUF = list(range(16, 32)) + list(range(0, 16))
    NDT = D // 128  # 8 d-tiles
    NST = L // 128  # 8 kv s-tiles

    with tile.TileContext(nc) as tc:
        with (
            nc.allow_low_precision(reason="float32r matmul pipeline"),
            tc.tile_pool(name="const", bufs=1) as cpool,
            tc.tile_pool(name="sb", bufs=2) as sb,
            tc.tile_pool(name="ps", bufs=2, space="PSUM") as ps,
            tc.tile_pool(name="dram", bufs=1, space="DRAM") as dram,
        ):
            # ---- constants ----
            patches_sb = cpool.tile([64, T], f32r, name="patches_sb")
            nc.sync.dma_start(out=patches_sb, in_=patchesT_d[:, :])
            cos_sb = cpool.tile([128, T], f32, name="cos_sb")
            nc.sync.dma_start(out=cos_sb, in_=cos_d[:, :])
            sinm_sb = cpool.tile([128, T], f32, name="sinm_sb")
            nc.sync.dma_start(out=sinm_sb, in_=sinm_d[:, :])
            emat2_sb = cpool.tile([2, 128], f32r, name="emat2_sb")
            nc.sync.dma_start(out=emat2_sb, in_=emat2_d[:, :])
            ones_col = cpool.tile([128, 1], f32r, name="ones_col")
            nc.vector.memset(ones_col[:].bitcast(f32), 1.0)
            ones_row = cpool.tile([1, 128], f32r, name="ones_row")
            nc.vector.memset(ones_row[:].bitcast(f32), 1.0)
            eps_t = cpool.tile([1, 1], f32, name="eps_t")
            nc.vector.memset(eps_t, EPS)

            # ---- persistent tok tiles ----
            tok = [cpool.tile([128, T], f32, name=f"tok{i}") for i in range(NDT)]

            # partition-id derived shard row offsets in cc_out
            pid = nc.sync.partition_id()
            row0 = (pid // 2) * 4096  # even shard base row ([16384, 512] layout)
            row0h = (pid // 2) * 2048  # same in row-pair units

            # ---- layer 0 input: tok = merge_w @ patches ----
            for og in range(2):
                mw = sb.tile([64, 512], f32r, tag="wg", bufs=WG_B, name="mw")
                nc.sync.dma_start(out=mw, in_=merge_wT_d[:, 512 * og : 512 * (og + 1)])
                for j in range(4):
                    ot = 4 * og + j
                    acc = ps.tile([128, T], f32, tag="acc", bufs=ACC_B, name="m_acc")
                    nc.tensor.matmul(
                        acc, mw[:, 128 * j : 128 * (j + 1)], patches_sb[:],
                        start=True, stop=True,
                    )
                    nc.vector.tensor_copy(out=tok[ot], in_=acc)

            def rmsnorm_h(out_dtype=f32r, out_tag="h", bufs=H_B):
                """h = tok * rsqrt(mean(tok^2, d) + eps), all in layout B."""
                _sctag = "sc2" if os.environ.get("VIT_PAIREXP") == "1" else "sc"
                ssq = ps.tile([1, T], f32, tag=_sctag, bufs=SC_B, name="ssq")
                _sqeng = nc.gpsimd if os.environ.get("VIT_GPS") == "1" else nc.vector
                for dt in range(NDT):
                    sq = sb.tile([128, T], f32r, tag="sq", bufs=SQ_B, name="sq")
                    _sqeng.tensor_mul(sq, tok[dt], tok[dt])
                    nc.tensor.matmul(
                        ssq, ones_col[:], sq[:], start=(dt == 0), stop=(dt == NDT - 1)
                    )
                srow = sb.tile([1, T], f32, tag="srow", bufs=2, name="srow")
                nc.scalar.activation(
                    out=srow, in_=ssq, func=AF.Sqrt, bias=eps_t[:], scale=1.0 / D
                )
                rrow = sb.tile([1, T], f32r, tag="rrow", bufs=2, name="rrow")
                nc.vector.reciprocal(out=rrow, in_=srow)
                bc = ps.tile([128, T], f32, tag=_sctag, bufs=SC_B, name="bc")
                nc.tensor.matmul(bc, ones_row[:], rrow[:], start=True, stop=True)
                hs = []
                for dt in range(NDT):
                    ht = sb.tile([128, T], out_dtype, tag=f"{out_tag}{dt}", bufs=bufs,
                                 name="ht")
                    nc.vector.tensor_mul(ht, tok[dt], bc)
                    hs.append(ht)
                return hs

            def project(w_slice, act, n_ct, n_ot, out_cb):
                """out[ot] = sum_ct w_slice(ct, og)[:, j].T @ act[ct]; groups of 4."""
                wg_prev = []
                for og in range(n_ot // 4):
                    if os.environ.get("VIT_WONCE") == "1" and og > 0:
                        wg = wg_prev  # timing-only ablation: reuse stale weights
                    else:
                        wg = []
                        for ct in range(n_ct):
                            wt = sb.tile([128, 512], f32r, tag="wg", bufs=WG_B, name="wt")
                            nc.sync.dma_start(out=wt, in_=w_slice(ct, og))
                            wg.append(wt)
                        wg_prev = wg
                    for j in range(4):
                        acc = ps.tile([128, T], f32, tag="acc", bufs=ACC_B, name="p_acc")
                        for ct in range(n_ct):
                            nc.tensor.matmul(
                                acc, wg[ct][:, 128 * j : 128 * (j + 1)], act[ct][:],
                                start=(ct == 0), stop=(ct == n_ct - 1),
                            )
                        out_cb(og * 4 + j, acc)

            def rope(acc, dst):
                """dst = acc*cos + shuffle(acc)*sinm (dst f32r)."""
                t1 = sb.tile([128, T], f32, tag="rt1", bufs=RT1_B, name="rt1")
                nc.vector.stream_shuffle(out=t1[:], in_=acc[:], mask=SHUF)
                t2 = sb.tile([128, T], f32, tag="rt2", bufs=RT2_B, name="rt2")
                nc.vector.tensor_mul(t2, t1, sinm_sb)
                t3 = sb.tile([128, T], f32, tag="rt3", bufs=RT3_B, name="rt3")
                nc.vector.tensor_mul(t3, acc, cos_sb)
                nc.vector.tensor_add(dst, t2, t3)

            for lyr in range(DEPTH):
                # ---------- attention half ----------
                h = rmsnorm_h()

                cc_in = dram.tile([2048, 512], f32, tag="cci", bufs=2, name="cc_in")
                cc_out = dram.tile(
                    [NCORES * 2048, 512], f32,
                    addr_space="Local" if os.environ.get("VIT_NOCC") == "1" else "Shared",
                    bufs=1, name=f"cc_out{lyr}",
                )
                cc_v = cc_in[:].rearrange("(r two) c -> r two c", two=2)
                cco_v = cc_out[:].rearrange("(r two) c -> r two c", two=2)

                # k projection + rope + stage (cols [1024, 2048) of qkvT)
                def k_cb(ot, acc):
                    kt = sb.tile([128, T], f32r, tag="kst", bufs=KST_B, name="kt")
                    rope(acc, kt)
                    nc.sync.dma_start(
                        out=cc_in[128 * ot : 128 * (ot + 1), :],
                        in_=kt[:].bitcast(f32),
                    )

                project(
                    lambda ct, og: qkvT_d[
                        lyr, 128 * ct : 128 * (ct + 1), D + 512 * og : D + 512 * (og + 1)
                    ],
                    h, NDT, NDT, k_cb,
                )

                # v projection + stage: v[t, o] with t on partitions (layout A)
                for nch in range(2):
                    wv = []
                    for dt in range(NDT):
                        wt = sb.tile([128, 512], f32r, tag="wg", bufs=WG_B, name="wvt")
                        nc.sync.dma_start(
                            out=wt,
                            in_=qkvT_d[
                                lyr, 128 * dt : 128 * (dt + 1),
                                2 * D + 512 * nch : 2 * D + 512 * (nch + 1),
                            ],
                        )
                        wv.append(wt)
                    for tt in range(4):
                        acc = ps.tile([128, T], f32, tag="acc", bufs=ACC_B, name="v_acc")
                        for dt in range(NDT):
                            nc.tensor.matmul(
                                acc, h[dt][:, 128 * tt : 128 * (tt + 1)], wv[dt][:],
                                start=(dt == 0), stop=(dt == NDT - 1),
                            )
                        vt = sb.tile([128, T], f32r, tag="kst", bufs=KST_B, name="vt")
                        nc.vector.tensor_copy(out=vt, in_=acc)
                        nc.sync.dma_start(
                            out=cc_v[512 + 128 * tt : 512 + 128 * (tt + 1), nch, :],
                            in_=vt[:].bitcast(f32),
                        )

                if os.environ.get("VIT_NOCC") == "1":
                    # timing/simulation variant: fake the gather with local DMAs
                    nc.sync.dma_start(out=cc_out[0:2048, :], in_=cc_in[:])
                    nc.sync.dma_start(out=cc_out[2048:4096, :], in_=cc_in[:])
                else:
                    nc.gpsimd.collective_compute(
                        "AllGather",
                        ALU.bypass,
                        replica_groups=[list(range(NCORES))],
                        ins=[cc_in[:].opt()],
                        outs=[cc_out[:].opt()],
                    )

                # q projection + rope (overlaps the all-gather)
                q_tiles = [None] * NDT

                def q_cb(ot, acc):
                    qt = sb.tile([128, T], f32r, tag=f"q{ot}", bufs=1, name="qt")
                    rope(acc, qt)
                    q_tiles[ot] = qt

                def emit_q():
                    project(
                        lambda ct, og: qkvT_d[
                            lyr, 128 * ct : 128 * (ct + 1), 512 * og : 512 * (og + 1)
                        ],
                        h, NDT, NDT, q_cb,
                    )

                if os.environ.get("VIT_VFIRST", "1") != "1":
                    emit_q()

                # v readback into v_aug [128, 16*65] tiles (ones col per head)
                vf = []
                for st in range(NST):
                    vt = sb.tile([128, HEADS * 65], f32r, tag=f"vf{st}", bufs=1,
                                 name="vft")
                    vv = vt[:].rearrange("p (h j) -> p h j", j=65)
                    half_off = 0 if st < 4 else 1024  # odd shard, in row-pair units
                    for nch in range(2):
                        nc.sync.dma_start(
                            out=vv[:, 8 * nch : 8 * (nch + 1), 0:64].bitcast(f32),
                            in_=cco_v[
                                ds(row0h + half_off + 512 + 128 * (st % 4), 128),
                                nch, :,
                            ],
                        )
                    nc.vector.memset(vv[:, :, 64:65].bitcast(f32), 1.0)
                    vf.append(vt)

                if os.environ.get("VIT_VFIRST", "1") == "1":
                    emit_q()

                # attention per head-pair
                o_tiles = [None] * NDT
                if os.environ.get("VIT_NOATT") == "1":
                    o_tiles = h  # timing-only ablation
                for hp in range(NDT * (0 if os.environ.get("VIT_NOATT") == "1" else 1)):
                    kf0 = sb.tile([128, 512], f32r, tag="kf0", bufs=KF_B, name="kf0")
                    nc.sync.dma_start(
                        out=kf0[:].bitcast(f32),
                        in_=cc_out[ds(row0 + 128 * hp, 128), :],
                    )
                    kf1 = sb.tile([128, 512], f32r, tag="kf1", bufs=KF_B, name="kf1")
                    nc.sync.dma_start(
                        out=kf1[:].bitcast(f32),
                        in_=cc_out[ds(row0 + 2048 + 128 * hp, 128), :],
                    )
                    kfh = [kf0, kf1]
                    rd = sb.tile([1, 2 * T], f32r, tag="rr2", bufs=2, name="rd")
                    oaccs = []
                    if os.environ.get("VIT_PAIREXP") == "1":
                        oaccs = [
                            ps.tile([65, T], f32, tag="oacc", bufs=OACC_B,
                                    name=f"oacc{hh}")
                            for hh in range(2)
                        ]
                        for st in range(NST):
                            sc = ps.tile([128, 2 * T], f32, tag="sc2", bufs=SC_B,
                                         name="sc")
                            for hh in range(2):
                                nc.tensor.matmul(
                                    sc[:, T * hh : T * (hh + 1)],
                                    kfh[st // 4][
                                        64 * hh : 64 * hh + 64,
                                        128 * (st % 4) : 128 * (st % 4 + 1),
                                    ],
                                    q_tiles[hp][64 * hh : 64 * hh + 64, :],
                                    start=True, stop=True,
                                )
                            pt = sb.tile([128, 2 * T], f32r, tag="p", bufs=P_B,
                                         name="pt")
                            nc.scalar.activation(
                                out=pt, in_=sc, func=AF.Exp, scale=1.0 / np.sqrt(DH)
                            )
                            for hh in range(2):
                                nc.tensor.matmul(
                                    oaccs[hh],
                                    vf[st][:, 65 * (2 * hp + hh) : 65 * (2 * hp + hh) + 65],
                                    pt[:, T * hh : T * (hh + 1)],
                                    start=(st == 0), stop=(st == NST - 1),
                                )
                        for hh in range(2):
                            nc.vector.reciprocal(
                                out=rd[0:1, T * hh : T * (hh + 1)],
                                in_=oaccs[hh][64:65, :],
                            )
                    else:
                        for hh in range(2):
                            habs = 2 * hp + hh
                            oacc = ps.tile([65, T], f32, tag="oacc", bufs=OACC_B, name="oacc")
                            for st in range(NST):
                                sc = ps.tile([128, T], f32, tag="sc", bufs=SC_B, name="sc")
                                nc.tensor.matmul(
                                    sc,
                                    kfh[st // 4][
                                        64 * hh : 64 * hh + 64,
                                        128 * (st % 4) : 128 * (st % 4 + 1),
                                    ],
                                    q_tiles[hp][64 * hh : 64 * hh + 64, :],
                                    start=True, stop=True,
                                )
                                pt = sb.tile([128, T], f32r, tag="p", bufs=P_B, name="pt")
                                nc.scalar.activation(
                                    out=pt, in_=sc, func=AF.Exp, scale=1.0 / np.sqrt(DH)
                                )
                                nc.tensor.matmul(
                                    oacc,
                                    vf[st][:, 65 * habs : 65 * habs + 65],
                                    pt[:],
                                    start=(st == 0), stop=(st == NST - 1),
                                )
                            nc.vector.reciprocal(
                                out=rd[0:1, T * hh : T * (hh + 1)], in_=oacc[64:65, :]
                            )
                            oaccs.append(oacc)
                    rr_d = dram.tile([2, T], f32, tag="rrd", bufs=2, name="rr_d")
                    nc.sync.dma_start(
                        out=rr_d[:].rearrange("p t -> (p t)"),
                        in_=rd[0:1, 0 : 2 * T].bitcast(f32),
                    )
                    rr2 = sb.tile([2, T], f32r, tag="rr2b", bufs=2, name="rr2")
                    nc.sync.dma_start(out=rr2[:].bitcast(f32), in_=rr_d[:])
                    rb = ps.tile(
                        [128, T], f32,
                        tag="sc2" if os.environ.get("VIT_PAIREXP") == "1" else "sc",
                        bufs=SC_B, name="rb",
                    )
                    nc.tensor.matmul(rb, emat2_sb[:], rr2[:], start=True, stop=True)
                    rb_sb = sb.tile([128, T], f32, tag="rt2", bufs=RT2_B, name="rb_sb")
                    nc.vector.tensor_copy(out=rb_sb, in_=rb)
                    ot_t = sb.tile([128, T], f32r, tag=f"big{hp}", bufs=1, name="ot_t")
                    nc.vector.tensor_mul(
                        ot_t[0:64, :], oaccs[0][0:64, :], rb_sb[0:64, :]
                    )
                    nc.vector.tensor_mul(
                        ot_t[64:128, :], oaccs[1][0:64, :], rb_sb[64:128, :]
                    )
                    o_tiles[hp] = ot_t

                # proj + residual
                def proj_cb(dt, acc):
                    nc.vector.tensor_add(tok[dt], tok[dt], acc)

                project(
                    lambda ct, og: projT_d[
                        lyr, 128 * ct : 128 * (ct + 1), 512 * og : 512 * (og + 1)
                    ],
                    o_tiles, NDT, NDT, proj_cb,
                )

                # ---------- MLP half ----------
                if os.environ.get("VIT_NOMLP") == "1":
                    continue
                h2 = rmsnorm_h()
                delta = [None] * NDT
                for qr in range(4):
                    hid = [None] * 8

                    def fc1_cb(j, acc, qr=qr, hid=hid):
                        gt = sb.tile([128, T], f32r, tag=f"hid{j}", bufs=1, name="gt")
                        nc.scalar.activation(
                            out=gt, in_=acc, func=AF.Gelu_apprx_tanh
                        )
                        hid[j] = gt

                    project(
                        lambda ct, og, qr=qr: fc1T_d[
                            lyr, 128 * ct : 128 * (ct + 1),
                            1024 * qr + 512 * og : 1024 * qr + 512 * (og + 1),
                        ],
                        h2, NDT, 8, fc1_cb,
                    )

                    def fc2_cb(dt, acc, qr=qr, hid=hid):
                        if qr == 0:
                            dl = sb.tile([128, T], f32, tag=f"big{dt}", bufs=1,
                                         name="dl")
                            nc.vector.tensor_copy(out=dl, in_=acc)
                            delta[dt] = dl
                        else:
                            nc.vector.tensor_add(delta[dt], delta[dt], acc)
                            if qr == 3:
                                eng = (nc.gpsimd if os.environ.get("VIT_GPS") == "1"
                                       else nc.vector)
                                eng.tensor_add(tok[dt], tok[dt], delta[dt])

                    project(
                        lambda ct, og, qr=qr: fc2T_d[
                            lyr, 1024 * qr + 128 * ct : 1024 * qr + 128 * (ct + 1),
                            512 * og : 512 * (og + 1),
                        ],
                        hid, 8, NDT, fc2_cb,
                    )

            # final rmsnorm -> z
            zs = rmsnorm_h(out_dtype=f32, out_tag="zz", bufs=1)
            for dt in range(NDT):
                nc.sync.dma_start(
                    out=z_d[128 * dt : 128 * (dt + 1), :], in_=zs[dt][:]
                )

    nc.compile()
    return nc


def _host_inputs(x, merge_w, qkv_w, proj_w, fc1_w, fc2_w):
    x = np.asarray(x, np.float32)
    Hf = Wf = 32
    patches = (
        x.reshape(NB, 16, Hf, FACT, Wf, FACT)
        .transpose(0, 2, 4, 1, 3, 5)
        .reshape(NB, L, 16 * FACT * FACT)
    )
    qkvT = np.ascontiguousarray(np.asarray(qkv_w, np.float32).transpose(0, 2, 1))
    projT = np.ascontiguousarray(np.asarray(proj_w, np.float32).transpose(0, 2, 1))
    fc1T = np.ascontiguousarray(np.asarray(fc1_w, np.float32).transpose(0, 2, 1))
    fc2T = np.ascontiguousarray(np.asarray(fc2_w, np.float32).transpose(0, 2, 1))
    merge_wT = np.ascontiguousarray(np.asarray(merge_w, np.float32).T)

    # rope tables per token-half: rows r in [0,128) = 2 heads x 64 head-dims
    inv_freq = (1.0 / (10000.0 ** (np.arange(16, dtype=np.float32) / 16.0))).astype(
        np.float32
    )
    cos_t = np.empty((2, 128, T), np.float32)
    sinm_t = np.empty((2, 128, T), np.float32)
    for half in range(2):
        t_glob = np.arange(half * T, (half + 1) * T, dtype=np.float32)
        pos_h = np.floor(t_glob / Wf).astype(np.float32)
        pos_w = (t_glob % Wf).astype(np.float32)
        for r in range(128):
            rr = r % 64
            pos = pos_h if rr < 32 else pos_w
            j = rr % 16
            ang = pos * inv_freq[j]
            cos_t[half, r] = np.cos(ang)
            s = np.sin(ang)
            sinm_t[half, r] = -s if (rr % 32) < 16 else s

    emat2 = np.zeros((2, 128), np.float32)
    emat2[0, 0:64] = 1.0
    emat2[1, 64:128] = 1.0

    in_maps = []
    for c in range(NCORES):
        n, half = c // 2, c % 2
        patchesT = np.ascontiguousarray(patches[n, half * T : (half + 1) * T, :].T)
        in_maps.append(
            {
                "patchesT": patchesT,
                "merge_wT": merge_wT,
                "qkvT": qkvT[:DEPTH],
                "projT": projT[:DEPTH],
                "fc1T": fc1T[:DEPTH],
                "fc2T": fc2T[:DEPTH],
                "cos_t": np.ascontiguousarray(cos_t[half]),
                "sinm_t": np.ascontiguousarray(sinm_t[half]),
                "emat2": emat2,
            }
        )
    return in_maps


def kernel(
    x, merge_w, qkv_w, qkv_b, proj_w, proj_b, fc1_w, fc1_b, fc2_w, fc2_b
) -> np.ndarray:
    from concourse.bass_utils import run_bass_kernel_spmd

    # biases are structurally zero for this model; the device kernel omits them
    for b in (qkv_b, proj_b, fc1_b, fc2_b):
        assert not np.any(np.asarray(b)), "nonzero biases unsupported"

    if "nc" not in _cache:
        _cache["nc"] = _build()
    nc = _cache["nc"]

    in_maps = _host_inputs(x, merge_w, qkv_w, proj_w, fc1_w, fc2_w)
    res = run_bass_kernel_spmd(nc, in_maps, core_ids=list(range(NCORES)))
    z = np.empty((NB, D, L), np.float32)
    for c in range(NCORES):
        n, half = c // 2, c % 2
        z[n, :, half * T : (half + 1) * T] = res.results[c]["z"]
    return z



# revision 52
# speedup vs baseline: 1.3165x; 1.3165x over previous
"""Trainium2 Bass kernel for nn_ModelA_ViT: 8-layer ViT encoder (D=1024, 16 heads,
2D RoPE, RMSNorm, GELU-tanh MLP) over 4x16x64x64 input, output [4, 1024, 1024].

Sharding: sequence-parallel over (batch, token-half): core c owns batch c//2,
tokens [512*(c%2), 512*(c%2)+512). Per layer, each core computes q/k/v for its
512 tokens, all-gathers rope'd K and V (fp8) across all 8 cores through DRAM,
and runs attention rows for its own tokens against the full 1024-token K/V of
its batch (shards selected with partition-id-based dynamic DMA offsets).

Precision: the big matmuls run as fp8(e4m3) DoubleRow pairs (2x128 contraction
per instruction at 0.5 cycles/row = 4x the f32r rate). Weights/activations are
scaled into fp8 range (powers of 2 folded into adjacent constants) and split
hi/lo: qkv/proj/fc1/fc2 use the 3-term scheme W@x ~= Whi@xhi + Wlo@xhi +
Whi@xlo (~0.1% rel err per matmul); attention q/k/p/v are single fp8 (softmax
washes the noise; scores use plain fp8 matmuls at K=64, P@V uses DoubleRow
over s-chunk pairs with a ones column for free softmax denominators).
Residual stream (tok) stays f32; RMSNorm stats are exact.

Layout: residual stream is [D on partitions (8x128), tokens on free (512)]
end-to-end. Weight slabs load as one [128, 8K] fp8 DMA per 512-col output
group (hi and lo halves side by side, DoubleRow-packed). Elementwise work is
spread across DVE / Activation / GPSIMD to keep all four engines busy;
residual adds use fused scalar_tensor_tensor ops.
"""

import sys

sys.path.insert(0, "/opt/trn_rl_repo")

import os

import numpy as np
import ml_dtypes

D = 1024
HEADS = 16
DH = 64
DEPTH = int(os.environ.get("VIT_LAYERS", "8"))
HID = 4096
NB = 4
L = 1024
T = 512  # tokens per core
NCORES = 8
EPS = 1e-6
FACT = 2
E4 = ml_dtypes.float8_e4m3

SX = 8.0      # activation scale for rmsnorm'd h (qkv, fc1 inputs)
SW = 256.0    # weight scale (all weight matrices)
SO = 64.0     # attention-output scale (proj input)
SQK = 32.0    # q/k scale after rope
SP = 2.0      # softmax p scale
SV = 32.0     # v scale

_cache = {}

def _env(name, dflt):
    return int(os.environ.get(name, str(dflt)))

WG_B = _env("VIT_WG", 4)
ACC_B = _env("VIT_ACC", 2)
SC_B = _env("VIT_SC", 2)
P_B = _env("VIT_P", 3)
KST_B = _env("VIT_KST", 2)
SQ_B = _env("VIT_SQ", 2)
RT1_B = _env("VIT_RT1", 2)
RT2_B = _env("VIT_RT2", 2)
RT3_B = _env("VIT_RT3", 2)
OACC_B = _env("VIT_OACC", 2)
H_B = _env("VIT_H", 1)
G_B = _env("VIT_G", 2)
HS_B = _env("VIT_HS", 2)
KF_B = _env("VIT_KF", 2)
FC2T = _env("VIT_FC2T", 3)


def _build():
    import concourse.bass as bass
    import concourse.bacc as bacc
    import concourse.mybir as mybir
    import concourse.tile as tile

    f32 = mybir.dt.float32
    f32r = mybir.dt.float32r
    f8 = mybir.dt.float8e4
    AF = mybir.ActivationFunctionType
    ALU = mybir.AluOpType
    DR = mybir.MatmulPerfMode.DoubleRow
    ds = bass.ds

    nc = bacc.Bacc("TRN2", target_bir_lowering=False, debug=False, num_devices=NCORES)

    patchesT_d = nc.dram_tensor("patchesT", [64, T], f32r, kind="ExternalInput")
    merge_wT_d = nc.dram_tensor("merge_wT", [64, D], f32r, kind="ExternalInput")
    # fp8 weight slabs: [layer, out-group(512 cols), 128, hi(4c x 1024) | lo(...)]
    qkv_w8_d = nc.dram_tensor("qkv_w8", [DEPTH, 6, 128, 8192], f8, kind="ExternalInput")
    proj_w8_d = nc.dram_tensor("proj_w8", [DEPTH, 2, 128, 8192], f8, kind="ExternalInput")
    fc1_w8_d = nc.dram_tensor("fc1_w8", [DEPTH, 8, 128, 8192], f8, kind="ExternalInput")
    fc2_w8_d = nc.dram_tensor("fc2_w8", [DEPTH, 8, 128, 8192], f8, kind="ExternalInput")
    emat16_d = nc.dram_tensor("emat16", [8, 512], f32r, kind="ExternalInput")
    cos_d = nc.dram_tensor("cos_t", [128, T], f32, kind="ExternalInput")
    sinm_d = nc.dram_tensor("sinm_t", [128, T], f32, kind="ExternalInput")
    z_d = nc.dram_tensor("z", [D, T], f32, kind="ExternalOutput")

    SHUF = list(range(16, 32)) + list(range(0, 16))
    NDT = D // 128  # 8 d-tiles
    NC2 = D // 256  # 4 DoubleRow k-chunks

    with tile.TileContext(nc) as tc:
        with (
            nc.allow_low_precision(reason="fp8 doublerow matmul pipeline"),
            tc.tile_pool(name="const", bufs=1) as cpool,
            tc.tile_pool(name="sb", bufs=2) as sb,
            tc.tile_pool(name="ps", bufs=2, space="PSUM") as ps,
            tc.tile_pool(name="dram", bufs=1, space="DRAM") as dram,
        ):
            # ---- constants ----
            patches_sb = cpool.tile([64, T], f32r, name="patches_sb")
            nc.sync.dma_start(out=patches_sb, in_=patchesT_d[:, :])
            cos_sb = cpool.tile([128, T], f32, name="cos_sb")
            nc.sync.dma_start(out=cos_sb, in_=cos_d[:, :])
            sinm_sb = cpool.tile([128, T], f32, name="sinm_sb")
            nc.sync.dma_start(out=sinm_sb, in_=sinm_d[:, :])
            emat16_sb = cpool.tile([8, 512], f32r, name="emat16_sb")
            nc.sync.dma_start(out=emat16_sb, in_=emat16_d[:, :])
            ones_col = cpool.tile([128, 1], f32r, name="ones_col")
            nc.vector.memset(ones_col[:].bitcast(f32), 1.0)
            ones_row = cpool.tile([1, 128], f32r, name="ones_row")
            nc.vector.memset(ones_row[:].bitcast(f32), 1.0)
            eps64_t = cpool.tile([1, 1], f32, name="eps64_t")
            nc.vector.memset(eps64_t, EPS / (SX * SX))
            eps_t = cpool.tile([1, 1], f32, name="eps_t")
            nc.vector.memset(eps_t, EPS)
            ln2_t = cpool.tile([128, 1], f32, name="ln2_t")
            nc.vector.memset(ln2_t, float(np.log(SP)))

            # ---- persistent tok tiles ----
            tok = [cpool.tile([128, T], f32, name=f"tok{i}") for i in range(NDT)]

            # persistent vf8 staging tiles: [128 s-part, 2 (st half), 16*65];
            # the ones column (softmax denominator) is set once and survives
            # the per-layer v readbacks, which only overwrite cols 0:64
            vf8 = []
            for c in range(4):
                vt = cpool.tile([128, 2 * HEADS * 65], f8, name=f"vf8_{c}")
                vv = vt[:].rearrange("p (two h j) -> p two h j", two=2, j=65)
                nc.vector.memset(vv[:, :, :, 64:65], 1.0)
                vf8.append(vv)

            # partition-id derived shard row offsets in cc_out
            pid = nc.sync.partition_id()
            row0k = (pid // 2) * 2048  # batch base row in cck_out [8192, 512]
            row0vh = (pid // 2) * 1024  # batch base in ccv_out row-pair units

            # ---- layer 0 input: tok = merge_w @ patches (f32r, tiny) ----
            for og in range(2):
                mw = sb.tile([64, 512], f32r, tag="mw", bufs=2, name="mw")
                nc.sync.dma_start(out=mw, in_=merge_wT_d[:, 512 * og : 512 * (og + 1)])
                for j in range(4):
                    ot = 4 * og + j
                    acc = ps.tile([128, T], f32, tag="acc", bufs=ACC_B, name="m_acc")
                    nc.tensor.matmul(
                        acc, mw[:, 128 * j : 128 * (j + 1)], patches_sb[:],
                        start=True, stop=True,
                    )
                    nc.vector.tensor_copy(out=tok[ot], in_=acc)

            def rmsnorm_fp8(tag):
                """xhi=fp8(SX*h), xlo=fp8(SX*h - xhi), packed in d-chunk pairs."""
                ssq = ps.tile([1, T], f32, tag="sc", bufs=SC_B, name="ssq")
                for dt in range(NDT):
                    sq = sb.tile([128, T], f32r, tag="sq", bufs=SQ_B, name="sq")
                    nc.gpsimd.tensor_mul(sq, tok[dt], tok[dt])
                    nc.tensor.matmul(
                        ssq, ones_col[:], sq[:], start=(dt == 0), stop=(dt == NDT - 1)
                    )
                srow = sb.tile([1, T], f32, tag="srow", bufs=2, name="srow")
                nc.scalar.activation(
                    out=srow, in_=ssq, func=AF.Sqrt, bias=eps64_t[:],
                    scale=1.0 / (D * SX * SX)
                )
                rrow = sb.tile([1, T], f32r, tag="rrow", bufs=2, name="rrow")
                nc.vector.reciprocal(out=rrow, in_=srow)
                bc = ps.tile([128, T], f32, tag="sc", bufs=SC_B, name="bc")
                nc.tensor.matmul(bc, ones_row[:], rrow[:], start=True, stop=True)
                # all hi halves first (matmul hi-terms unblock early), lo after
                xhi, xlo, hss = [], [], []
                for c in range(NC2):
                    ht = sb.tile([128, 2 * T], f8, tag=f"{tag}h{c}", bufs=H_B, name="ht")
                    lt = sb.tile([128, 2 * T], f8, tag=f"{tag}l{c}", bufs=H_B, name="lt")
                    hv = ht[:].rearrange("p (two t) -> p two t", two=2)
                    lv = lt[:].rearrange("p (two t) -> p two t", two=2)
                    for i in range(2):
                        hs = sb.tile([128, T], f32, tag=f"hs{(2 * c + i) % 4}",
                                     bufs=HS_B, name="hs")
                        nc.vector.tensor_mul(hs, tok[2 * c + i], bc)
                        nc.gpsimd.tensor_copy(out=hv[:, i, :], in_=hs)
                        hss.append((hs, hv, lv, i))
                    xhi.append(hv)
                    xlo.append(lv)
                for hs, hv, lv, i in hss:
                    nc.vector.tensor_sub(lv[:, i, :], hs, hv[:, i, :])
                return xhi, xlo

            def load_slab(w_d, lyr, og):
                """One DMA: [128, 8192] = 4 hi chunks | 4 lo chunks, DR-packed."""
                wt = sb.tile([128, 8192], f8, tag="wg", bufs=WG_B, name="wt")
                nc.sync.dma_start(out=wt, in_=w_d[lyr, og])
                whi = [
                    wt[:, 1024 * c : 1024 * (c + 1)].rearrange(
                        "p (two m) -> p two m", two=2
                    )
                    for c in range(NC2)
                ]
                wlo = [
                    wt[:, 4096 + 1024 * c : 4096 + 1024 * (c + 1)].rearrange(
                        "p (two m) -> p two m", two=2
                    )
                    for c in range(NC2)
                ]
                return whi, wlo

            def project_dr(slabs, xhi, xlo, n_ot, out_cb, nterm=3):
                """out[ot] = fp8 DR: Whi@xhi [+ Wlo@xhi + Whi@xlo]."""
                for og in range(n_ot // 4):
                    whi, wlo = slabs[og]
                    for j in range(4):
                        acc = ps.tile([128, T], f32, tag="acc", bufs=ACC_B, name="p_acc")
                        njs = slice(128 * j, 128 * (j + 1))
                        seq = []
                        for c in range(NC2):
                            seq.append((whi[c][:, :, njs], xhi[c]))
                        for c in range(NC2):
                            seq.append((wlo[c][:, :, njs], xhi[c]))
                            if nterm >= 3:
                                seq.append((whi[c][:, :, njs], xlo[c]))
                        for idx, (wv, xv) in enumerate(seq):
                            nc.tensor.matmul(
                                acc, wv, xv, start=(idx == 0),
                                stop=(idx == len(seq) - 1), perf_mode=DR,
                            )
                        out_cb(og * 4 + j, acc)

            def rope(acc, dst, alt=[0]):
                """dst = acc*cos' + shuffle(acc)*sinm' (fp8 out, scales folded).

                DVE does the shuffle; the two muls and the add alternate
                between DVE and GPSIMD so neither engine falls behind the PE.
                """
                alt[0] ^= 1
                e_add = nc.vector if alt[0] else nc.gpsimd
                t1 = sb.tile([128, T], f32, tag="rt1", bufs=RT1_B, name="rt1")
                nc.vector.stream_shuffle(out=t1[:], in_=acc[:], mask=SHUF)
                t2 = sb.tile([128, T], f32, tag="rt2", bufs=RT2_B, name="rt2")
                nc.gpsimd.tensor_mul(t2, t1, sinm_sb)
                t3 = sb.tile([128, T], f32, tag="rt3", bufs=RT3_B, name="rt3")
                nc.vector.tensor_mul(t3, acc, cos_sb)
                e_add.tensor_add(dst, t2, t3)

            for lyr in range(DEPTH):
                # ---------- attention half ----------
                # k slabs first (pure loads); q/v slabs issue mid-stream so no
                # SP-queue WAR wait blocks them behind data-dependent DMAs
                slabs_k = [load_slab(qkv_w8_d, lyr, 2 + og) for og in range(2)]

                xhi, xlo = rmsnorm_fp8("a")

                # split k / v staging buffers: two AllGathers so the k-gather
                # overlaps the q/v projections and the v-gather overlaps the
                # score matmuls (which need only k and q)
                cck_in = dram.tile([1024, 512], f8, tag="ccki", bufs=2, name="cck_in")
                ccv_in = dram.tile([1024, 512], f8, tag="ccvi", bufs=2, name="ccv_in")
                nocc = os.environ.get("VIT_NOCC") == "1"
                cck_out = dram.tile(
                    [NCORES * 1024, 512], f8,
                    addr_space="Local" if nocc else "Shared",
                    bufs=1, name=f"cck_out{lyr}",
                )
                ccv_out = dram.tile(
                    [NCORES * 1024, 512], f8,
                    addr_space="Local" if nocc else "Shared",
                    bufs=1, name=f"ccv_out{lyr}",
                )
                ccv_vi = ccv_in[:].rearrange("(r two) c -> r two c", two=2)
                ccov = ccv_out[:].rearrange("(r two) c -> r two c", two=2)

                # k projection + rope + stage (out-groups 2,3 of qkv)
                def k_cb(ot, acc):
                    kt = sb.tile([128, T], f8, tag="kst", bufs=KST_B, name="kt")
                    rope(acc, kt)
                    nc.sync.dma_start(
                        out=cck_in[128 * ot : 128 * (ot + 1), :], in_=kt[:],
                    )

                project_dr(slabs_k, xhi, xlo, NDT, k_cb)

                # q/v slabs load now (pure loads, before any waiting DMA)
                slabs_q = [load_slab(qkv_w8_d, lyr, og) for og in range(2)]
                slabs_v = [load_slab(qkv_w8_d, lyr, 4 + og) for og in range(2)]

                # k gather: overlaps the q and v projections below
                if nocc:
                    nc.sync.dma_start(out=cck_out[0:1024, :], in_=cck_in[:])
                    nc.sync.dma_start(out=cck_out[1024:2048, :], in_=cck_in[:])
                else:
                    nc.gpsimd.collective_compute(
                        "AllGather",
                        ALU.bypass,
                        replica_groups=[list(range(NCORES))],
                        ins=[cck_in[:].opt()],
                        outs=[cck_out[:].opt()],
                    )

                # k readbacks (need only the k gather) issue before any
                # v-staging waits can block the SP queue
                kf_pre = []
                for hp in range(2):
                    kf0 = sb.tile([128, 512], f8, tag="kf0", bufs=KF_B, name="kf0")
                    nc.sync.dma_start(
                        out=kf0[:], in_=cck_out[ds(row0k + 128 * hp, 128), :],
                    )
                    kf1 = sb.tile([128, 512], f8, tag="kf1", bufs=KF_B, name="kf1")
                    nc.sync.dma_start(
                        out=kf1[:], in_=cck_out[ds(row0k + 1024 + 128 * hp, 128), :],
                    )
                    kf_pre.append((kf0, kf1))

                # v projection + stage: v[t, o] with t on partitions
                for nch in range(2):
                    wvh, wvl = slabs_v[nch]
                    for tt in range(4):
                        acc = ps.tile([128, T], f32, tag="acc", bufs=ACC_B, name="v_acc")
                        tts = slice(128 * tt, 128 * (tt + 1))
                        seq = []
                        for c in range(NC2):
                            seq.append((xhi[c][:, :, tts], wvh[c]))
                        for c in range(NC2):
                            seq.append((xlo[c][:, :, tts], wvh[c]))
                            seq.append((xhi[c][:, :, tts], wvl[c]))
                        for idx, (sv, mv) in enumerate(seq):
                            nc.tensor.matmul(
                                acc, sv, mv, start=(idx == 0),
                                stop=(idx == len(seq) - 1), perf_mode=DR,
                            )
                        vt = sb.tile([128, T], f8, tag="kst", bufs=KST_B, name="vt")
                        nc.scalar.activation(
                            out=vt, in_=acc, func=AF.Copy, scale=SV / (SX * SW)
                        )
                        nc.sync.dma_start(
                            out=ccv_vi[128 * tt : 128 * (tt + 1), nch, :],
                            in_=vt[:],
                        )

                # v gather: overlaps the q projection and score matmuls
                if nocc:
                    nc.sync.dma_start(out=ccv_out[0:1024, :], in_=ccv_in[:])
                    nc.sync.dma_start(out=ccv_out[1024:2048, :], in_=ccv_in[:])
                else:
                    nc.gpsimd.collective_compute(
                        "AllGather",
                        ALU.bypass,
                        replica_groups=[list(range(NCORES))],
                        ins=[ccv_in[:].opt()],
                        outs=[ccv_out[:].opt()],
                    )

                # q projection + rope -> fp8 tiles kept in SBUF (overlaps the
                # v gather; scores follow immediately in the PE stream)
                q_tiles = [None] * NDT

                def q_cb(ot, acc):
                    qt = sb.tile([128, T], f8, tag=f"q{ot}", bufs=1, name="qt")
                    rope(acc, qt)
                    q_tiles[ot] = qt

                project_dr(slabs_q, xhi, xlo, NDT, q_cb)

                # v readback into the persistent vf8 tiles (cols 0:64 only)
                for c in range(4):
                    vv = vf8[c]
                    for half in range(2):
                        st = 2 * c + half
                        half_off = 512 if st >= 4 else 0  # shard, row-pair units
                        for nch in range(2):
                            nc.sync.dma_start(
                                out=vv[:, half, 8 * nch : 8 * (nch + 1), 0:64],
                                in_=ccov[
                                    ds(row0vh + half_off + 128 * (st % 4), 128),
                                    nch, :,
                                ],
                            )

                # proj slabs: pure loads, queued behind the gather-dependent
                # readbacks but needed only after the whole attention block
                slabs_p = [load_slab(proj_w8_d, lyr, og) for og in range(2)]

                # attention per head-pair: fp8 scores (K=64) + DR PV.
                # numerators/denominators evacuate to SBUF per head; softmax
                # normalization happens once for all 16 heads (single DRAM
                # bounce) so the PV/score pipeline never stalls on it.
                o_tiles = [None] * NDT
                rrecs = []
                pts = {}
                rr_d = dram.tile([16, T], f32, tag="rrd", bufs=2, name="rr_d")

                def emit_scores(hp):
                    """scores + paired exps for head-pair hp; pt tiles kept."""
                    if hp < 2:
                        kfh = kf_pre[hp]
                    else:
                        kf0 = sb.tile([128, 512], f8, tag="kf0", bufs=KF_B,
                                      name="kf0")
                        nc.sync.dma_start(
                            out=kf0[:], in_=cck_out[ds(row0k + 128 * hp, 128), :],
                        )
                        kf1 = sb.tile([128, 512], f8, tag="kf1", bufs=KF_B,
                                      name="kf1")
                        nc.sync.dma_start(
                            out=kf1[:],
                            in_=cck_out[ds(row0k + 1024 + 128 * hp, 128), :],
                        )
                        kfh = (kf0, kf1)
                    for hh in range(2):
                        for c in range(4):
                            pt = sb.tile([128, 2 * T], f8, tag=f"p{hp % 2}{hh}{c}",
                                         bufs=1, name="pt")
                            ptv = pt[:].rearrange("p (two t) -> p two t", two=2)
                            sc = ps.tile([128, 2 * T], f32, tag="sc", bufs=SC_B,
                                         name="sc")
                            for half in range(2):
                                st = 2 * c + half
                                nc.tensor.matmul(
                                    sc[:, T * half : T * (half + 1)],
                                    kfh[st // 4][
                                        64 * hh : 64 * hh + 64,
                                        128 * (st % 4) : 128 * (st % 4 + 1),
                                    ],
                                    q_tiles[hp][64 * hh : 64 * hh + 64, :],
                                    start=True, stop=True,
                                )
                            nc.scalar.activation(
                                out=pt[:], in_=sc, func=AF.Exp,
                                bias=ln2_t[:],
                                scale=1.0 / (SQK * SQK * np.sqrt(DH)),
                            )
                            pts[(hp, hh, c)] = ptv

                def emit_pv(hp):
                    ot_t = sb.tile([128, T], f32, tag=f"big{hp}", bufs=1,
                                   name="ot_t")
                    for hh in range(2):
                        ha = 2 * hp + hh
                        oacc = ps.tile([65, T], f32, tag="oacc", bufs=OACC_B,
                                       name="oacc")
                        for c in range(4):
                            nc.tensor.matmul(
                                oacc,
                                vf8[c][:, :, ha, :],
                                pts.pop((hp, hh, c)),
                                start=(c == 0), stop=(c == 3), perf_mode=DR,
                            )
                        nc.vector.tensor_copy(
                            out=ot_t[64 * hh : 64 * hh + 64, :], in_=oacc[0:64, :],
                        )
                        den_t = sb.tile([1, T], f32, tag="den", bufs=2, name="den")
                        nc.vector.tensor_copy(out=den_t, in_=oacc[64:65, :])
                        nc.sync.dma_start(out=rr_d[ha : ha + 1, :], in_=den_t[:])
                    o_tiles[hp] = ot_t
                    if hp == 3 or hp == 7:
                        # bounce this half's denominators now: the readback +
                        # reciprocal overlap the remaining head-pairs
                        base = 0 if hp == 3 else 8
                        rrh = sb.tile([8, T], f32, tag="rr16", bufs=2, name="rr16")
                        nc.sync.dma_start(out=rrh[:], in_=rr_d[base : base + 8, :])
                        rrec = sb.tile([8, T], f32r, tag="rrec", bufs=2,
                                       name="rrec")
                        nc.vector.reciprocal(out=rrec, in_=rrh)
                        rrecs.append(rrec)

                # scores run two head-pairs ahead of PV so the PV never
                # stalls the in-order PE stream on the v readback chain
                emit_scores(0)
                emit_scores(1)
                for hp in range(2, NDT):
                    emit_scores(hp)
                    emit_pv(hp - 2)
                emit_pv(NDT - 2)
                emit_pv(NDT - 1)

                # per head-pair: broadcast 1/den and scale o in place
                ohi, olo = [None] * NC2, [None] * NC2
                for half in range(2):
                    for hp in range(4 * half, 4 * half + 4):
                        rb = ps.tile([128, T], f32, tag="sc", bufs=SC_B, name="rb")
                        nc.tensor.matmul(
                            rb,
                            emat16_sb[:, 128 * (hp % 4) : 128 * (hp % 4 + 1)],
                            rrecs[half][:],
                            start=True, stop=True,
                        )
                        rb_sb = sb.tile([128, T], f32, tag="rt2", bufs=RT2_B,
                                        name="rb_sb")
                        nc.vector.tensor_copy(out=rb_sb, in_=rb)
                        nc.gpsimd.tensor_mul(
                            o_tiles[hp][:], o_tiles[hp][:], rb_sb[:]
                        )
                    # o hi/lo packed pairs for 3-term proj (o at SO scale)
                    for c in (2 * half, 2 * half + 1):
                        ht = sb.tile([128, 2 * T], f8, tag=f"oh{c}", bufs=1,
                                     name="ohi")
                        lt = sb.tile([128, 2 * T], f8, tag=f"ol{c}", bufs=1,
                                     name="olo")
                        hv = ht[:].rearrange("p (two t) -> p two t", two=2)
                        lv = lt[:].rearrange("p (two t) -> p two t", two=2)
                        for i in range(2):
                            src = o_tiles[2 * c + i]
                            nc.gpsimd.tensor_copy(out=hv[:, i, :], in_=src[:])
                            nc.vector.tensor_sub(lv[:, i, :], src[:], hv[:, i, :])
                        ohi[c] = hv
                        olo[c] = lv

                # proj + residual (fused scale+add)
                def proj_cb(dt, acc):
                    nc.vector.scalar_tensor_tensor(
                        out=tok[dt], in0=acc, scalar=1.0 / (SO * SW), in1=tok[dt],
                        op0=ALU.mult, op1=ALU.add,
                    )

                project_dr(slabs_p, ohi, olo, NDT, proj_cb)

                # ---------- MLP half ----------
                xhi2, xlo2 = rmsnorm_fp8("m")
                delta = [None] * NDT
                for qr in range(4):
                    slabs_f1 = [load_slab(fc1_w8_d, lyr, 2 * qr + og)
                                for og in range(2)]
                    slabs_f2 = [load_slab(fc2_w8_d, lyr, 2 * qr + og)
                                for og in range(2)]
                    ghi = [None] * 4
                    glo = [None] * 4

                    def fc1_cb(j, acc, qr=qr, ghi=ghi, glo=glo):
                        c, i = j // 2, j % 2
                        if ghi[c] is None:
                            ht = sb.tile([128, 2 * T], f8, tag=f"gh{qr % 2}{c}",
                                         bufs=1, name="ghi")
                            ghi[c] = ht[:].rearrange("p (two t) -> p two t", two=2)
                            if FC2T >= 3:
                                lt = sb.tile([128, 2 * T], f8, tag=f"gl{qr % 2}{c}",
                                             bufs=1, name="glo")
                                glo[c] = lt[:].rearrange("p (two t) -> p two t", two=2)
                        if FC2T >= 3:
                            g32 = sb.tile([128, T], f32, tag="g32", bufs=G_B,
                                          name="g32")
                            nc.scalar.activation(
                                out=g32, in_=acc, func=AF.Gelu_apprx_tanh,
                                scale=1.0 / (SX * SW),
                            )
                            nc.gpsimd.tensor_copy(out=ghi[c][:, i, :], in_=g32)
                            nc.vector.tensor_sub(glo[c][:, i, :], g32, ghi[c][:, i, :])
                        else:
                            nc.scalar.activation(
                                out=ghi[c][:, i, :], in_=acc, func=AF.Gelu_apprx_tanh,
                                scale=1.0 / (SX * SW),
                            )

                    project_dr(slabs_f1, xhi2, xlo2, 8, fc1_cb)

                    def fc2_cb(dt, acc, qr=qr):
                        if qr == 0:
                            dl = sb.tile([128, T], f32, tag=f"big{dt}", bufs=1,
                                         name="dl")
                            nc.vector.tensor_copy(out=dl, in_=acc)
                            delta[dt] = dl
                        elif qr < 3:
                            nc.vector.tensor_add(delta[dt], delta[dt], acc)
                        else:
                            nc.vector.tensor_add(delta[dt], delta[dt], acc)
                            nc.vector.scalar_tensor_tensor(
                                out=tok[dt], in0=delta[dt], scalar=1.0 / SW,
                                in1=tok[dt], op0=ALU.mult, op1=ALU.add,
                            )

                    project_dr(slabs_f2, ghi, glo, NDT, fc2_cb, nterm=FC2T)

            # final rmsnorm -> z (exact, f32)
            ssq = ps.tile([1, T], f32, tag="sc", bufs=SC_B, name="fssq")
            for dt in range(NDT):
                sq = sb.tile([128, T], f32r, tag="sq", bufs=SQ_B, name="fsq")
                nc.vector.tensor_mul(sq, tok[dt], tok[dt])
                nc.tensor.matmul(
                    ssq, ones_col[:], sq[:], start=(dt == 0), stop=(dt == NDT - 1)
                )
            srow = sb.tile([1, T], f32, tag="srow", bufs=2, name="fsrow")
            nc.scalar.activation(
                out=srow, in_=ssq, func=AF.Sqrt, bias=eps_t[:], scale=1.0 / D
            )
            rrow = sb.tile([1, T], f32r, tag="rrow", bufs=2, name="frrow")
            nc.vector.reciprocal(out=rrow, in_=srow)
            bc = ps.tile([128, T], f32, tag="sc", bufs=SC_B, name="fbc")
            nc.tensor.matmul(bc, ones_row[:], rrow[:], start=True, stop=True)
            for dt in range(NDT):
                zt = sb.tile([128, T], f32, tag=f"hs{dt % 4}", bufs=HS_B, name="zt")
                nc.vector.tensor_mul(zt, tok[dt], bc)
                nc.sync.dma_start(
                    out=z_d[128 * dt : 128 * (dt + 1), :], in_=zt[:]
                )

    nc.compile()
    return nc


def _pack_slabs(wT, scale):
    """[K, M] f32 -> [M//512, 128, 8192] fp8 slabs (hi 4c | lo 4c), DR-packed."""
    K, M = wT.shape
    ws = (wT * scale).astype(np.float32)
    hi = ws.astype(E4)
    lo = (ws - hi.astype(np.float32)).astype(E4)

    def pack(a):
        # [K, M] -> [c, i, r, og, m] -> [og, r, c, i*512+m] -> [og, 128, c*1024]
        a = a.reshape(K // 256, 2, 128, M // 512, 512)
        a = a.transpose(3, 2, 0, 1, 4).reshape(M // 512, 128, (K // 256) * 1024)
        return a

    return np.ascontiguousarray(np.concatenate([pack(hi), pack(lo)], axis=-1))


def _host_inputs(x, merge_w, qkv_w, proj_w, fc1_w, fc2_w):
    x = np.asarray(x, np.float32)
    Hf = Wf = 32
    patches = (
        x.reshape(NB, 16, Hf, FACT, Wf, FACT)
        .transpose(0, 2, 4, 1, 3, 5)
        .reshape(NB, L, 16 * FACT * FACT)
    )
    merge_wT = np.ascontiguousarray(np.asarray(merge_w, np.float32).T)

    qkv_w8 = np.empty((DEPTH, 6, 128, 8192), E4)
    proj_w8 = np.empty((DEPTH, 2, 128, 8192), E4)
    fc1_w8 = np.empty((DEPTH, 8, 128, 8192), E4)
    fc2_w8 = np.empty((DEPTH, 8, 128, 8192), E4)
    for i in range(DEPTH):
        qkv_w8[i] = _pack_slabs(np.asarray(qkv_w[i], np.float32).T, SW)
        proj_w8[i] = _pack_slabs(np.asarray(proj_w[i], np.float32).T, SW)
        fc1_w8[i] = _pack_slabs(np.asarray(fc1_w[i], np.float32).T, SW)
        # fc2: [4096, 1024]: per-qr 1024-row bands -> slabs indexed 2*qr+og
        wT = np.asarray(fc2_w[i], np.float32).T
        for qr in range(4):
            fc2_w8[i, 2 * qr : 2 * qr + 2] = _pack_slabs(
                wT[1024 * qr : 1024 * (qr + 1)], SW
            )

    # rope tables: rows r in [0,128) = 2 heads x 64 head-dims.
    # folded scale: qkv psum descale (1/(SX*SW)) * fp8 q/k scale SQK.
    tab_scale = SQK / (SX * SW)
    inv_freq = (1.0 / (10000.0 ** (np.arange(16, dtype=np.float32) / 16.0))).astype(
        np.float32
    )
    cos_t = np.empty((2, 128, T), np.float32)
    sinm_t = np.empty((2, 128, T), np.float32)
    for half in range(2):
        t_glob = np.arange(half * T, (half + 1) * T, dtype=np.float32)
        pos_h = np.floor(t_glob / Wf).astype(np.float32)
        pos_w = (t_glob % Wf).astype(np.float32)
        for r in range(128):
            rr = r % 64
            pos = pos_h if rr < 32 else pos_w
            j = rr % 16
            ang = pos * inv_freq[j]
            cos_t[half, r] = np.cos(ang) * tab_scale
            s = np.sin(ang) * tab_scale
            sinm_t[half, r] = -s if (rr % 32) < 16 else s

    # oacc[0:64] = SV*SP*sum(p v); denom row = SP*sum(p).
    # ot = oacc * emat/denom = SV*emat*o_true; want SO*o_true -> emat = SO/SV.
    # emat16[k, 128*hpp + r] selects head k = 2*hpp + r//64 (per 8-head half).
    emat16 = np.zeros((8, 512), np.float32)
    for hpp in range(4):
        emat16[2 * hpp, 128 * hpp : 128 * hpp + 64] = SO / SV
        emat16[2 * hpp + 1, 128 * hpp + 64 : 128 * hpp + 128] = SO / SV

    in_maps = []
    for c in range(NCORES):
        n, half = c // 2, c % 2
        patchesT = np.ascontiguousarray(patches[n, half * T : (half + 1) * T, :].T)
        in_maps.append(
            {
                "patchesT": patchesT,
                "merge_wT": merge_wT,
                "qkv_w8": qkv_w8,
                "proj_w8": proj_w8,
                "fc1_w8": fc1_w8,
                "fc2_w8": fc2_w8,
                "cos_t": np.ascontiguousarray(cos_t[half]),
                "sinm_t": np.ascontiguousarray(sinm_t[half]),
                "emat16": emat16,
            }
        )
    return in_maps


def kernel(
    x, merge_w, qkv_w, qkv_b, proj_w, proj_b, fc1_w, fc1_b, fc2_w, fc2_b
) -> np.ndarray:
    from concourse.bass_utils import run_bass_kernel_spmd

    # biases are structurally zero for this model; the device kernel omits them
    for b in (qkv_b, proj_b, fc1_b, fc2_b):
        assert not np.any(np.asarray(b)), "nonzero biases unsupported"

    if "nc" not in _cache:
        _cache["nc"] = _build()
    nc = _cache["nc"]

    in_maps = _host_inputs(x, merge_w, qkv_w, proj_w, fc1_w, fc2_w)
    res = run_bass_kernel_spmd(nc, in_maps, core_ids=list(range(NCORES)))
    z = np.empty((NB, D, L), np.float32)
    for c in range(NCORES):
        n, half = c // 2, c % 2
        z[n, :, half * T : (half + 1) * T] = res.results[c]["z"]
    return z
